# revision 9
# baseline (speedup 1.0000x reference)
"""Trainium2 Bass kernel for nn_Cache_68135361184561 (retrieval_knn).

Computation (per batch element b, bsz=8):
    q_b   = query[0, :, b, :]                      # [L=64, h=1024]
    k_b   = keys[:, b, :].reshape(128, 64, 1024)   # [N, L, h]
    att[b, n] = max_{i,j} q_b[i] . k_b[n, j]       # [128]
    topk_idx  = top-8 blocks by att

values (512 MB) is unused by the reference computation.

Sharding: batch b -> NeuronCore b (8 cores, fully batch-parallel).

Device kernel (per core), 16 groups of 8 n-blocks each:
  - one 2MB DMA per group; partition p = (n_oct, j_hi) so each partition
    reads 16KB contiguous HBM (4 consecutive j-rows) -> fat descriptors
  - PE-transposes 128x128 chunks (exact fp32) into 2-bank PSUM tiles
  - DVE/ACT evict PSUM -> SBUF, reordering columns n-major and rounding
    to float32r (PE requirement for full-rate 4-byte matmul)
  - score S[i, j'] over 8 h-chunk matmuls (lhsT = qT chunk [128h, 64i],
    rhs = K^T [128h, 512] f32r, fp32 PSUM accumulate)
  - DVE max-reduce S per n-block, accumulate [64 i, 128 n]
  - final PE transpose + DVE max over i -> att [128 n]

Host: gathers per-core att, re-ranks top-16 candidate blocks in fp64 from
the raw inputs to produce exact top-8 indices.
"""
from contextlib import ExitStack

import numpy as np

TOPK = 8
TOPC = 16  # candidate blocks re-ranked on host in fp64
N_CORES = 8
L = 64
H = 1024
N_BLOCKS = 128
DK = L * H  # 65536
GROUPS = 16  # each group covers 8 n-blocks

_CACHE = {}


def _build():
    import concourse.bacc as bacc
    import concourse.tile as tile
    import concourse.mybir as mybir
    from concourse import masks

    F32 = mybir.dt.float32
    F32R = mybir.dt.float32r

    nc = bacc.Bacc("TRN2", target_bir_lowering=False, debug=False)
    q = nc.dram_tensor("q", [L, H], F32, kind="ExternalInput").ap()
    keys = nc.dram_tensor("keys", [N_BLOCKS, DK], F32, kind="ExternalInput").ap()
    att = nc.dram_tensor("att", [N_BLOCKS, 1], F32, kind="ExternalOutput").ap()

    with tile.TileContext(nc) as tc, ExitStack() as ctx:
        kn_pool = ctx.enter_context(tc.tile_pool(name="kn", bufs=4))
        kt_pool = ctx.enter_context(tc.tile_pool(name="kt", bufs=3))
        small = ctx.enter_context(tc.tile_pool(name="small", bufs=1))
        pt_pool = ctx.enter_context(tc.tile_pool(name="pt", bufs=2, space="PSUM"))
        ps_pool = ctx.enter_context(tc.tile_pool(name="ps", bufs=2, space="PSUM"))
        aux_pool = ctx.enter_context(tc.tile_pool(name="aux", bufs=2, space="PSUM"))

        ident = small.tile([128, 128], F32, tag="ident")
        masks.make_identity(nc, ident[:])

        # query: load natural [64, 1024], PE-transpose 8 chunks -> qTr
        # [128 h, (c, i)] rounded to f32r
        qn = small.tile([L, H], F32, tag="qn")
        nc.sync.dma_start(qn[:], q)
        qTr = small.tile([128, 8 * L], F32R, tag="qTr")
        for c in range(8):
            qp = aux_pool.tile([128, L], F32, tag="aux")
            nc.tensor.matmul(
                qp[:], qn[:, 128 * c : 128 * (c + 1)], ident[0:L, 0:L],
                is_transpose=True,
            )
            nc.vector.tensor_copy(qTr[:, L * c : L * (c + 1)], qp[:])

        # per-i accumulated block maxima [64 i, 128 n]
        att_acc = small.tile([L, N_BLOCKS], F32, tag="att_acc")

        for g in range(GROUPS):
            # one 2MB DMA per group; kn partition p = (n_oct, j_hi),
            # free = (j_lo, h): 16KB contiguous per partition
            kn = kn_pool.tile([128, 4 * H], F32, tag="kn")
            dma_eng = nc.sync if g % 2 == 0 else nc.scalar
            dma_eng.dma_start(
                kn[:].rearrange("p (jl h) -> p jl h", h=H),
                keys[8 * g : 8 * g + 8, :].rearrange(
                    "n (jh jl h) -> (n jh) jl h", jl=4, h=H
                ),
            )

            s_ps = ps_pool.tile([L, 512], F32, tag="s")
            for m in range(4):  # pairs of h-chunks
                pt = pt_pool.tile([128, 1024], F32, tag="pt")
                for c2 in range(2):
                    c = 2 * m + c2
                    for l in range(4):
                        # transpose kn[(n j_hi), (j_lo=l, h-chunk c)]
                        nc.tensor.matmul(
                            pt[:, 512 * c2 + 128 * l : 512 * c2 + 128 * (l + 1)],
                            kn[:, 1024 * l + 128 * c : 1024 * l + 128 * (c + 1)],
                            ident[:],
                            is_transpose=True,
                        )
                # evict 2 banks at once (contiguous copy, rounds to f32r)
                kt = kt_pool.tile([128, 1024], F32R, tag="kt")
                if m % 2 == 0:
                    nc.vector.tensor_copy(kt[:], pt[:])
                else:
                    nc.scalar.copy(kt[:], pt[:])
                for c2 in range(2):
                    c = 2 * m + c2
                    nc.tensor.matmul(
                        s_ps[:],
                        qTr[:, L * c : L * (c + 1)],
                        kt[:, 512 * c2 : 512 * (c2 + 1)],
                        start=(c == 0),
                        stop=(c == 7),
                    )

            # s_ps columns are (l 4, n 8, j_hi 16): reduce max over j_hi,
            # then over l, keeping the 8 n-blocks
            red1 = small.tile([L, 32], F32, tag="red1")
            nc.vector.reduce_max(
                red1[:],
                s_ps[:].rearrange("i (ln j) -> i ln j", j=16),
                axis=mybir.AxisListType.X,
            )
            nc.vector.reduce_max(
                att_acc[:, 8 * g : 8 * (g + 1)],
                red1[:].rearrange("i (l n) -> i n l", l=4),
                axis=mybir.AxisListType.X,
            )

        # final: transpose [64 i, 128 n] -> [128 n, 64 i], max over i
        pfin = aux_pool.tile([128, L], F32, tag="aux")
        nc.tensor.matmul(
            pfin[:], att_acc[:], ident[0:L, 0:L], is_transpose=True
        )
        fin = small.tile([N_BLOCKS, 1], F32, tag="fin")
        nc.vector.reduce_max(fin[:], pfin[:], axis=mybir.AxisListType.X)
        nc.sync.dma_start(att, fin[:])

    nc.compile()
    return nc


def _get_nc():
    if "nc" not in _CACHE:
        _CACHE["nc"] = _build()
    return _CACHE["nc"]


def kernel(query: np.ndarray, keys: np.ndarray, values: np.ndarray):
    from concourse import bass_utils

    assert query.shape == (1, L, N_CORES, H)
    assert keys.shape == (N_BLOCKS, N_CORES, DK)

    nc = _get_nc()

    in_maps = []
    for b in range(N_CORES):
        qb = np.ascontiguousarray(query[0, :, b, :], dtype=np.float32)  # [L, H]
        kb = np.ascontiguousarray(keys[:, b, :], dtype=np.float32)  # [N, DK]
        in_maps.append({"q": qb, "keys": kb})

    res = bass_utils.run_bass_kernel_spmd(
        nc, in_maps, core_ids=list(range(N_CORES)), **_CACHE.get("run_kwargs", {})
    )
    _CACHE["last_result"] = res

    att = np.empty((N_CORES, 1, N_BLOCKS), dtype=np.float32)
    for b in range(N_CORES):
        att[b, 0, :] = res.results[b]["att"][:, 0]

    # exact top-k: re-rank top candidate blocks in fp64 from raw inputs
    topk = np.empty((TOPK, N_CORES), dtype=np.int32)
    for b in range(N_CORES):
        cand = np.argsort(-att[b, 0], kind="stable")[:TOPC]
        qb = query[0, :, b, :].astype(np.float64)  # [64, 1024]
        kb = keys[cand, b, :].reshape(TOPC, L, H).astype(np.float64)
        # scores[n] = max_{i,j} q[i] . k[n, j]
        s = np.einsum("ih,njh->nij", qb, kb, optimize=True)
        sc = s.reshape(TOPC, -1).max(axis=1)
        order = np.argsort(-sc, kind="stable")[:TOPK]
        topk[:, b] = cand[order].astype(np.int32)

    return att, topk


# revision 10
# speedup vs baseline: 1.0216x; 1.0216x over previous
"""Trainium2 Bass kernel for nn_Cache_68135361184561 (retrieval_knn).

Computation (per batch element b, bsz=8):
    q_b   = query[0, :, b, :]                      # [L=64, h=1024]
    k_b   = keys[:, b, :].reshape(128, 64, 1024)   # [N, L, h]
    att[b, n] = max_{i,j} q_b[i] . k_b[n, j]       # [128]
    topk_idx  = top-8 blocks by att

values (512 MB) is unused by the reference computation.

Sharding: batch b -> NeuronCore b (8 cores, fully batch-parallel).

Device kernel (per core), 16 groups of 8 n-blocks each:
  - one 2MB DMA per group; partition p = (n_oct, j_hi) so each partition
    reads 16KB contiguous HBM (4 consecutive j-rows) -> fat descriptors
  - PE-transposes 128x128 chunks (exact fp32) into 2-bank PSUM tiles
  - DVE/ACT evict PSUM -> SBUF, reordering columns n-major and rounding
    to float32r (PE requirement for full-rate 4-byte matmul)
  - score S[i, j'] over 8 h-chunk matmuls (lhsT = qT chunk [128h, 64i],
    rhs = K^T [128h, 512] f32r, fp32 PSUM accumulate)
  - DVE max-reduce S per n-block, accumulate [64 i, 128 n]
  - final PE transpose + DVE max over i -> att [128 n]

Host: gathers per-core att, re-ranks top-16 candidate blocks in fp64 from
the raw inputs to produce exact top-8 indices.
"""
from contextlib import ExitStack

import numpy as np

TOPK = 8
TOPC = 16  # candidate blocks re-ranked on host in fp64
N_CORES = 8
L = 64
H = 1024
N_BLOCKS = 128
DK = L * H  # 65536
GROUPS = 16  # each group covers 8 n-blocks

_CACHE = {}


def _build():
    import concourse.bacc as bacc
    import concourse.tile as tile
    import concourse.mybir as mybir
    from concourse import masks

    F32 = mybir.dt.float32
    F32R = mybir.dt.float32r

    nc = bacc.Bacc("TRN2", target_bir_lowering=False, debug=False)
    q = nc.dram_tensor("q", [L, H], F32, kind="ExternalInput").ap()
    keys = nc.dram_tensor("keys", [N_BLOCKS, DK], F32, kind="ExternalInput").ap()
    att = nc.dram_tensor("att", [N_BLOCKS, 1], F32, kind="ExternalOutput").ap()

    with tile.TileContext(nc) as tc, ExitStack() as ctx:
        kn_pool = ctx.enter_context(tc.tile_pool(name="kn", bufs=6))
        kt_pool = ctx.enter_context(tc.tile_pool(name="kt", bufs=3))
        small = ctx.enter_context(tc.tile_pool(name="small", bufs=1))
        pt_pool = ctx.enter_context(tc.tile_pool(name="pt", bufs=2, space="PSUM"))
        ps_pool = ctx.enter_context(tc.tile_pool(name="ps", bufs=2, space="PSUM"))
        aux_pool = ctx.enter_context(tc.tile_pool(name="aux", bufs=1, space="PSUM"))

        # issue the first keys DMAs before anything else so the SDMA
        # engines start streaming immediately
        kns = {}

        def load_group(g):
            kn = kn_pool.tile([128, 4 * H], F32, tag="kn")
            dma_eng = nc.sync if g % 2 == 0 else nc.scalar
            dma_eng.dma_start(
                kn[:].rearrange("p (jl h) -> p jl h", h=H),
                keys[8 * g : 8 * g + 8, :].rearrange(
                    "n (jh jl h) -> (n jh) jl h", jl=4, h=H
                ),
            )
            kns[g] = kn

        PREFETCH = 4
        for g in range(PREFETCH):
            load_group(g)

        ident = small.tile([128, 128], F32, tag="ident")
        masks.make_identity(nc, ident[:])

        # query: load natural [64, 1024], PE-transpose 8 chunks -> qTr
        # [128 h, (c, i)] rounded to f32r
        qn = small.tile([L, H], F32, tag="qn")
        nc.scalar.dma_start(qn[:], q)
        qTr = small.tile([128, 8 * L], F32R, tag="qTr")
        for c in range(8):
            qp = aux_pool.tile([128, L], F32, tag="aux")
            nc.tensor.matmul(
                qp[:], qn[:, 128 * c : 128 * (c + 1)], ident[0:L, 0:L],
                is_transpose=True,
            )
            nc.vector.tensor_copy(qTr[:, L * c : L * (c + 1)], qp[:])

        # per-i accumulated block maxima [64 i, 128 n]
        att_acc = small.tile([L, N_BLOCKS], F32, tag="att_acc")

        for g in range(GROUPS):
            if g + PREFETCH < GROUPS:
                load_group(g + PREFETCH)
            kn = kns.pop(g)

            s_ps = ps_pool.tile([L, 512], F32, tag="s")
            for m in range(4):  # pairs of h-chunks
                pt = pt_pool.tile([128, 1024], F32, tag="pt")
                for c2 in range(2):
                    c = 2 * m + c2
                    for l in range(4):
                        # transpose kn[(n j_hi), (j_lo=l, h-chunk c)]
                        nc.tensor.matmul(
                            pt[:, 512 * c2 + 128 * l : 512 * c2 + 128 * (l + 1)],
                            kn[:, 1024 * l + 128 * c : 1024 * l + 128 * (c + 1)],
                            ident[:],
                            is_transpose=True,
                        )
                # evict 2 banks at once (contiguous copy, rounds to f32r)
                kt = kt_pool.tile([128, 1024], F32R, tag="kt")
                if m % 2 == 0:
                    nc.vector.tensor_copy(kt[:], pt[:])
                else:
                    nc.scalar.copy(kt[:], pt[:])
                for c2 in range(2):
                    c = 2 * m + c2
                    nc.tensor.matmul(
                        s_ps[:],
                        qTr[:, L * c : L * (c + 1)],
                        kt[:, 512 * c2 : 512 * (c2 + 1)],
                        start=(c == 0),
                        stop=(c == 7),
                    )

            # s_ps columns are (l 4, n 8, j_hi 16): reduce max over j_hi,
            # then over l, keeping the 8 n-blocks
            red1 = small.tile([L, 32], F32, tag="red1")
            nc.vector.reduce_max(
                red1[:],
                s_ps[:].rearrange("i (ln j) -> i ln j", j=16),
                axis=mybir.AxisListType.X,
            )
            nc.vector.reduce_max(
                att_acc[:, 8 * g : 8 * (g + 1)],
                red1[:].rearrange("i (l n) -> i n l", l=4),
                axis=mybir.AxisListType.X,
            )

        # final: transpose [64 i, 128 n] -> [128 n, 64 i], max over i
        pfin = aux_pool.tile([128, L], F32, tag="aux")
        nc.tensor.matmul(
            pfin[:], att_acc[:], ident[0:L, 0:L], is_transpose=True
        )
        fin = small.tile([N_BLOCKS, 1], F32, tag="fin")
        nc.vector.reduce_max(fin[:], pfin[:], axis=mybir.AxisListType.X)
        nc.sync.dma_start(att, fin[:])

    nc.compile()
    return nc


def _get_nc():
    if "nc" not in _CACHE:
        _CACHE["nc"] = _build()
    return _CACHE["nc"]


def kernel(query: np.ndarray, keys: np.ndarray, values: np.ndarray):
    from concourse import bass_utils

    assert query.shape == (1, L, N_CORES, H)
    assert keys.shape == (N_BLOCKS, N_CORES, DK)

    nc = _get_nc()

    in_maps = []
    for b in range(N_CORES):
        qb = np.ascontiguousarray(query[0, :, b, :], dtype=np.float32)  # [L, H]
        kb = np.ascontiguousarray(keys[:, b, :], dtype=np.float32)  # [N, DK]
        in_maps.append({"q": qb, "keys": kb})

    res = bass_utils.run_bass_kernel_spmd(
        nc, in_maps, core_ids=list(range(N_CORES)), **_CACHE.get("run_kwargs", {})
    )
    _CACHE["last_result"] = res

    att = np.empty((N_CORES, 1, N_BLOCKS), dtype=np.float32)
    for b in range(N_CORES):
        att[b, 0, :] = res.results[b]["att"][:, 0]

    # exact top-k: re-rank top candidate blocks in fp64 from raw inputs
    topk = np.empty((TOPK, N_CORES), dtype=np.int32)
    for b in range(N_CORES):
        cand = np.argsort(-att[b, 0], kind="stable")[:TOPC]
        qb = query[0, :, b, :].astype(np.float64)  # [64, 1024]
        kb = keys[cand, b, :].reshape(TOPC, L, H).astype(np.float64)
        # scores[n] = max_{i,j} q[i] . k[n, j]
        s = np.einsum("ih,njh->nij", qb, kb, optimize=True)
        sc = s.reshape(TOPC, -1).max(axis=1)
        order = np.argsort(-sc, kind="stable")[:TOPK]
        topk[:, b] = cand[order].astype(np.int32)

    return att, topk


# revision 11
# speedup vs baseline: 1.0321x; 1.0102x over previous
"""Trainium2 Bass kernel for nn_Cache_68135361184561 (retrieval_knn).

Computation (per batch element b, bsz=8):
    q_b   = query[0, :, b, :]                      # [L=64, h=1024]
    k_b   = keys[:, b, :].reshape(128, 64, 1024)   # [N, L, h]
    att[b, n] = max_{i,j} q_b[i] . k_b[n, j]       # [128]
    topk_idx  = top-8 blocks by att

values (512 MB) is unused by the reference computation.

Sharding: batch b -> NeuronCore b (8 cores, fully batch-parallel).

Device kernel (per core), 16 groups of 8 n-blocks each:
  - one 2MB DMA per group; partition p = (n_oct, j_hi) so each partition
    reads 16KB contiguous HBM (4 consecutive j-rows) -> fat descriptors
  - PE-transposes 128x128 chunks (exact fp32) into 2-bank PSUM tiles
  - DVE/ACT evict PSUM -> SBUF, reordering columns n-major and rounding
    to float32r (PE requirement for full-rate 4-byte matmul)
  - score S[i, j'] over 8 h-chunk matmuls (lhsT = qT chunk [128h, 64i],
    rhs = K^T [128h, 512] f32r, fp32 PSUM accumulate)
  - DVE max-reduce S per n-block, accumulate [64 i, 128 n]
  - final PE transpose + DVE max over i -> att [128 n]

Host: gathers per-core att, re-ranks top-16 candidate blocks in fp64 from
the raw inputs to produce exact top-8 indices.
"""
from contextlib import ExitStack

import numpy as np

TOPK = 8
TOPC = 16  # candidate blocks re-ranked on host in fp64
N_CORES = 8
L = 64
H = 1024
N_BLOCKS = 128
DK = L * H  # 65536
GROUPS = 16  # each group covers 8 n-blocks

_CACHE = {}


def _build():
    import concourse.bacc as bacc
    import concourse.tile as tile
    import concourse.mybir as mybir
    from concourse import masks

    F32 = mybir.dt.float32
    F32R = mybir.dt.float32r

    nc = bacc.Bacc("TRN2", target_bir_lowering=False, debug=False)
    q = nc.dram_tensor("q", [L, H], F32, kind="ExternalInput").ap()
    keys = nc.dram_tensor("keys", [N_BLOCKS, DK], F32, kind="ExternalInput").ap()
    att = nc.dram_tensor("att", [N_BLOCKS, 1], F32, kind="ExternalOutput").ap()

    with tile.TileContext(nc) as tc, ExitStack() as ctx:
        kn_pool = ctx.enter_context(tc.tile_pool(name="kn", bufs=6))
        kt_pool = ctx.enter_context(tc.tile_pool(name="kt", bufs=3))
        small = ctx.enter_context(tc.tile_pool(name="small", bufs=1))
        pt_pool = ctx.enter_context(tc.tile_pool(name="pt", bufs=2, space="PSUM"))
        ps_pool = ctx.enter_context(tc.tile_pool(name="ps", bufs=2, space="PSUM"))
        aux_pool = ctx.enter_context(tc.tile_pool(name="aux", bufs=1, space="PSUM"))

        # tiny setup DMAs first (query on the scalar ring, ~0.6us), then
        # keys prefetch so the SDMA engines stream keys from the start
        qn = small.tile([L, H], F32, tag="qn")
        nc.scalar.dma_start(qn[:], q)
        ident = small.tile([128, 128], F32, tag="ident")
        masks.make_identity(nc, ident[:])

        kns = {}

        def load_group(g):
            kn = kn_pool.tile([128, 4 * H], F32, tag="kn")
            dma_eng = nc.sync if g % 2 == 0 else nc.scalar
            dma_eng.dma_start(
                kn[:].rearrange("p (jl h) -> p jl h", h=H),
                keys[8 * g : 8 * g + 8, :].rearrange(
                    "n (jh jl h) -> (n jh) jl h", jl=4, h=H
                ),
            )
            kns[g] = kn

        PREFETCH = 4
        for g in range(PREFETCH):
            load_group(g)

        # query: PE-transpose 8 chunks -> qTr [128 h, (c, i)], round to f32r
        qTr = small.tile([128, 8 * L], F32R, tag="qTr")
        for c in range(8):
            qp = aux_pool.tile([128, L], F32, tag="aux")
            nc.tensor.matmul(
                qp[:], qn[:, 128 * c : 128 * (c + 1)], ident[0:L, 0:L],
                is_transpose=True,
            )
            nc.vector.tensor_copy(qTr[:, L * c : L * (c + 1)], qp[:])

        # per-i accumulated block maxima [64 i, 128 n]
        att_acc = small.tile([L, N_BLOCKS], F32, tag="att_acc")

        for g in range(GROUPS):
            if g + PREFETCH < GROUPS:
                load_group(g + PREFETCH)
            kn = kns.pop(g)

            s_ps = ps_pool.tile([L, 512], F32, tag="s")
            for m in range(4):  # pairs of h-chunks
                pt = pt_pool.tile([128, 1024], F32, tag="pt")
                for c2 in range(2):
                    c = 2 * m + c2
                    for l in range(4):
                        # transpose kn[(n j_hi), (j_lo=l, h-chunk c)]
                        nc.tensor.matmul(
                            pt[:, 512 * c2 + 128 * l : 512 * c2 + 128 * (l + 1)],
                            kn[:, 1024 * l + 128 * c : 1024 * l + 128 * (c + 1)],
                            ident[:],
                            is_transpose=True,
                        )
                # evict 2 banks at once (contiguous copy, rounds to f32r)
                kt = kt_pool.tile([128, 1024], F32R, tag="kt")
                if m % 2 == 0:
                    nc.vector.tensor_copy(kt[:], pt[:])
                else:
                    nc.scalar.copy(kt[:], pt[:])
                for c2 in range(2):
                    c = 2 * m + c2
                    nc.tensor.matmul(
                        s_ps[:],
                        qTr[:, L * c : L * (c + 1)],
                        kt[:, 512 * c2 : 512 * (c2 + 1)],
                        start=(c == 0),
                        stop=(c == 7),
                    )

            # s_ps columns are (l 4, n 8, j_hi 16): reduce max over j_hi,
            # then over l, keeping the 8 n-blocks
            red1 = small.tile([L, 32], F32, tag="red1")
            nc.vector.reduce_max(
                red1[:],
                s_ps[:].rearrange("i (ln j) -> i ln j", j=16),
                axis=mybir.AxisListType.X,
            )
            nc.vector.reduce_max(
                att_acc[:, 8 * g : 8 * (g + 1)],
                red1[:].rearrange("i (l n) -> i n l", l=4),
                axis=mybir.AxisListType.X,
            )

        # final: transpose [64 i, 128 n] -> [128 n, 64 i], max over i
        pfin = aux_pool.tile([128, L], F32, tag="aux")
        nc.tensor.matmul(
            pfin[:], att_acc[:], ident[0:L, 0:L], is_transpose=True
        )
        fin = small.tile([N_BLOCKS, 1], F32, tag="fin")
        nc.vector.reduce_max(fin[:], pfin[:], axis=mybir.AxisListType.X)
        nc.sync.dma_start(att, fin[:])

    nc.compile()
    return nc


def _get_nc():
    if "nc" not in _CACHE:
        _CACHE["nc"] = _build()
    return _CACHE["nc"]


def kernel(query: np.ndarray, keys: np.ndarray, values: np.ndarray):
    from concourse import bass_utils

    assert query.shape == (1, L, N_CORES, H)
    assert keys.shape == (N_BLOCKS, N_CORES, DK)

    nc = _get_nc()

    in_maps = []
    for b in range(N_CORES):
        qb = np.ascontiguousarray(query[0, :, b, :], dtype=np.float32)  # [L, H]
        kb = np.ascontiguousarray(keys[:, b, :], dtype=np.float32)  # [N, DK]
        in_maps.append({"q": qb, "keys": kb})

    res = bass_utils.run_bass_kernel_spmd(
        nc, in_maps, core_ids=list(range(N_CORES)), **_CACHE.get("run_kwargs", {})
    )
    _CACHE["last_result"] = res

    att = np.empty((N_CORES, 1, N_BLOCKS), dtype=np.float32)
    for b in range(N_CORES):
        att[b, 0, :] = res.results[b]["att"][:, 0]

    # exact top-k: re-rank top candidate blocks in fp64 from raw inputs
    topk = np.empty((TOPK, N_CORES), dtype=np.int32)
    for b in range(N_CORES):
        cand = np.argsort(-att[b, 0], kind="stable")[:TOPC]
        qb = query[0, :, b, :].astype(np.float64)  # [64, 1024]
        kb = keys[cand, b, :].reshape(TOPC, L, H).astype(np.float64)
        # scores[n] = max_{i,j} q[i] . k[n, j]
        s = np.einsum("ih,njh->nij", qb, kb, optimize=True)
        sc = s.reshape(TOPC, -1).max(axis=1)
        order = np.argsort(-sc, kind="stable")[:TOPK]
        topk[:, b] = cand[order].astype(np.int32)

    return att, topk


# revision 16
# speedup vs baseline: 1.0763x; 1.0428x over previous
"""Trainium2 Bass kernel for nn_Cache_68135361184561 (retrieval_knn).

Computation (per batch element b, bsz=8):
    q_b   = query[0, :, b, :]                      # [L=64, h=1024]
    k_b   = keys[:, b, :].reshape(128, 64, 1024)   # [N, L, h]
    att[b, n] = max_{i,j} q_b[i] . k_b[n, j]       # [128]
    topk_idx  = top-8 blocks by att

values (512 MB) is unused by the reference computation.

Sharding: batch b -> NeuronCore b (8 cores, fully batch-parallel).

Device kernel (per core), 16 groups of 8 n-blocks each:
  - one 2MB DMA per group; partition p = (n_oct, j_hi) so each partition
    reads 16KB contiguous HBM (4 consecutive j-rows) -> fat descriptors
  - PE-transposes 128x128 chunks (exact fp32) into 2-bank PSUM tiles
  - DVE/ACT evict PSUM -> SBUF, reordering columns n-major and rounding
    to float32r (PE requirement for full-rate 4-byte matmul)
  - score S[i, j'] over 8 h-chunk matmuls (lhsT = qT chunk [128h, 64i],
    rhs = K^T [128h, 512] f32r, fp32 PSUM accumulate)
  - DVE max-reduce S per n-block, accumulate [64 i, 128 n]
  - final PE transpose + DVE max over i -> att [128 n]

Host: gathers per-core att, re-ranks top-16 candidate blocks in fp64 from
the raw inputs to produce exact top-8 indices.
"""
from contextlib import ExitStack

import numpy as np

TOPK = 8
TOPC = 16  # candidate blocks re-ranked on host in fp64
N_CORES = 8
L = 64
H = 1024
N_BLOCKS = 128
DK = L * H  # 65536
GROUPS = 16  # each group covers 8 n-blocks

_CACHE = {}


def _build():
    import concourse.bacc as bacc
    import concourse.tile as tile
    import concourse.mybir as mybir
    from concourse import masks

    F32 = mybir.dt.float32
    F32R = mybir.dt.float32r

    nc = bacc.Bacc("TRN2", target_bir_lowering=False, debug=False)
    # qp[p, c*64+i] = query[i, 128*c + p] (host-packed transposed query)
    qp = nc.dram_tensor("qp", [128, 8 * L], F32, kind="ExternalInput").ap()
    keys = nc.dram_tensor("keys", [N_BLOCKS, DK], F32, kind="ExternalInput").ap()
    att = nc.dram_tensor("att", [N_BLOCKS, 1], F32, kind="ExternalOutput").ap()

    with tile.TileContext(nc) as tc, ExitStack() as ctx:
        kn_pool = ctx.enter_context(tc.tile_pool(name="kn", bufs=6))
        kt_pool = ctx.enter_context(tc.tile_pool(name="kt", bufs=5))
        small = ctx.enter_context(tc.tile_pool(name="small", bufs=1))
        pt_pool = ctx.enter_context(tc.tile_pool(name="pt", bufs=5, space="PSUM"))
        ps_pool = ctx.enter_context(tc.tile_pool(name="ps", bufs=2, space="PSUM"))
        aux_pool = ctx.enter_context(tc.tile_pool(name="aux", bufs=1, space="PSUM"))

        # tiny setup DMA first (query on the scalar ring), then keys
        # prefetch so the SDMA engines stream keys from the start
        qsb = small.tile([128, 8 * L], F32, tag="qsb")
        nc.scalar.dma_start(qsb[:], qp)
        ident = small.tile([128, 128], F32, tag="ident")
        masks.make_identity(nc, ident[:])

        kns = {}

        def load_group(g, split=1):
            kn = kn_pool.tile([128, 4 * H], F32, tag="kn")
            dma_eng = nc.sync if g % 2 == 0 else nc.scalar
            src = keys[8 * g : 8 * g + 8, :].rearrange(
                "n (jh jl h) -> (n jh) jl h", jl=4, h=H
            )
            dst = kn[:].rearrange("p (jl h) -> p jl h", h=H)
            step = 4 // split
            for i in range(split):
                dma_eng.dma_start(
                    dst[:, i * step : (i + 1) * step],
                    src[:, i * step : (i + 1) * step],
                )
            kns[g] = kn

        PREFETCH = 4
        # split the first two groups so PE transposes can start on the
        # first quarter instead of waiting for a full 2MB tile
        load_group(0, split=4)
        load_group(1, split=2)
        for g in range(2, PREFETCH):
            load_group(g)

        qTr = small.tile([128, 8 * L], F32R, tag="qTr")
        nc.vector.tensor_copy(qTr[:], qsb[:])

        # per-i accumulated block maxima [64 i, 128 n]
        att_acc = small.tile([L, N_BLOCKS], F32, tag="att_acc")

        for g in range(GROUPS):
            if g + PREFETCH < GROUPS:
                load_group(g + PREFETCH)
            kn = kns.pop(g)

            s_ps = ps_pool.tile([L, 512], F32, tag="s")
            for c in range(8):  # h-chunks
                pt = pt_pool.tile([128, 512], F32, tag="pt")
                for l in range(4):
                    # transpose kn[(n j_hi), (j_lo=l, h-chunk c)]
                    nc.tensor.matmul(
                        pt[:, 128 * l : 128 * (l + 1)],
                        kn[:, 1024 * l + 128 * c : 1024 * l + 128 * (c + 1)],
                        ident[:],
                        is_transpose=True,
                    )
                # evict one bank (contiguous copy, rounds to f32r)
                kt = kt_pool.tile([128, 512], F32R, tag="kt")
                if c % 2 == 0:
                    nc.vector.tensor_copy(kt[:], pt[:])
                else:
                    nc.scalar.copy(kt[:], pt[:])
                nc.tensor.matmul(
                    s_ps[:],
                    qTr[:, L * c : L * (c + 1)],
                    kt[:],
                    start=(c == 0),
                    stop=(c == 7),
                )

            # s_ps columns are (l 4, n 8, j_hi 16): reduce max over j_hi,
            # then over l, keeping the 8 n-blocks
            red1 = small.tile([L, 32], F32, tag="red1")
            nc.vector.reduce_max(
                red1[:],
                s_ps[:].rearrange("i (ln j) -> i ln j", j=16),
                axis=mybir.AxisListType.X,
            )
            nc.vector.reduce_max(
                att_acc[:, 8 * g : 8 * (g + 1)],
                red1[:].rearrange("i (l n) -> i n l", l=4),
                axis=mybir.AxisListType.X,
            )

        # final: transpose [64 i, 128 n] -> [128 n, 64 i], max over i
        pfin = aux_pool.tile([128, L], F32, tag="aux")
        nc.tensor.matmul(
            pfin[:], att_acc[:], ident[0:L, 0:L], is_transpose=True
        )
        fin = small.tile([N_BLOCKS, 1], F32, tag="fin")
        nc.vector.reduce_max(fin[:], pfin[:], axis=mybir.AxisListType.X)
        nc.sync.dma_start(att, fin[:])

    nc.compile()
    return nc


def _get_nc():
    if "nc" not in _CACHE:
        _CACHE["nc"] = _build()
    return _CACHE["nc"]


def kernel(query: np.ndarray, keys: np.ndarray, values: np.ndarray):
    from concourse import bass_utils

    assert query.shape == (1, L, N_CORES, H)
    assert keys.shape == (N_BLOCKS, N_CORES, DK)

    nc = _get_nc()

    in_maps = []
    for b in range(N_CORES):
        qb = query[0, :, b, :]  # [L, H]
        # qp[p, c*64+i] = qb[i, 128c+p]
        qpk = np.ascontiguousarray(
            qb.T.reshape(8, 128, L).transpose(1, 0, 2).reshape(128, 8 * L),
            dtype=np.float32,
        )
        kb = np.ascontiguousarray(keys[:, b, :], dtype=np.float32)  # [N, DK]
        in_maps.append({"qp": qpk, "keys": kb})

    res = bass_utils.run_bass_kernel_spmd(
        nc, in_maps, core_ids=list(range(N_CORES)), **_CACHE.get("run_kwargs", {})
    )
    _CACHE["last_result"] = res

    att = np.empty((N_CORES, 1, N_BLOCKS), dtype=np.float32)
    for b in range(N_CORES):
        att[b, 0, :] = res.results[b]["att"][:, 0]

    # exact top-k: re-rank top candidate blocks in fp64 from raw inputs
    topk = np.empty((TOPK, N_CORES), dtype=np.int32)
    for b in range(N_CORES):
        cand = np.argsort(-att[b, 0], kind="stable")[:TOPC]
        qb = query[0, :, b, :].astype(np.float64)  # [64, 1024]
        kb = keys[cand, b, :].reshape(TOPC, L, H).astype(np.float64)
        # scores[n] = max_{i,j} q[i] . k[n, j]
        s = np.einsum("ih,njh->nij", qb, kb, optimize=True)
        sc = s.reshape(TOPC, -1).max(axis=1)
        order = np.argsort(-sc, kind="stable")[:TOPK]
        topk[:, b] = cand[order].astype(np.int32)

    return att, topk


# revision 18
# speedup vs baseline: 1.0874x; 1.0103x over previous
"""Trainium2 Bass kernel for nn_Cache_68135361184561 (retrieval_knn).

Computation (per batch element b, bsz=8):
    q_b   = query[0, :, b, :]                      # [L=64, h=1024]
    k_b   = keys[:, b, :].reshape(128, 64, 1024)   # [N, L, h]
    att[b, n] = max_{i,j} q_b[i] . k_b[n, j]       # [128]
    topk_idx  = top-8 blocks by att

values (512 MB) is unused by the reference computation.

Sharding: batch b -> NeuronCore b (8 cores, fully batch-parallel).

Device kernel (per core), 16 groups of 8 n-blocks each:
  - one 2MB DMA per group; partition p = (n_oct, j_hi) so each partition
    reads 16KB contiguous HBM (4 consecutive j-rows) -> fat descriptors
  - PE-transposes 128x128 chunks (exact fp32) into 2-bank PSUM tiles
  - DVE/ACT evict PSUM -> SBUF, reordering columns n-major and rounding
    to float32r (PE requirement for full-rate 4-byte matmul)
  - score S[i, j'] over 8 h-chunk matmuls (lhsT = qT chunk [128h, 64i],
    rhs = K^T [128h, 512] f32r, fp32 PSUM accumulate)
  - DVE max-reduce S per n-block, accumulate [64 i, 128 n]
  - final PE transpose + DVE max over i -> att [128 n]

Host: gathers per-core att, re-ranks top-16 candidate blocks in fp64 from
the raw inputs to produce exact top-8 indices.
"""
from contextlib import ExitStack

import numpy as np

TOPK = 8
TOPC = 16  # candidate blocks re-ranked on host in fp64
N_CORES = 8
L = 64
H = 1024
N_BLOCKS = 128
DK = L * H  # 65536
GROUPS = 16  # each group covers 8 n-blocks

_CACHE = {}


def _build():
    import concourse.bacc as bacc
    import concourse.tile as tile
    import concourse.mybir as mybir
    from concourse import masks

    F32 = mybir.dt.float32
    F32R = mybir.dt.float32r

    nc = bacc.Bacc("TRN2", target_bir_lowering=False, debug=False)
    # qp[p, c*64+i] = query[i, 128*c + p] (host-packed transposed query)
    qp = nc.dram_tensor("qp", [128, 8 * L], F32, kind="ExternalInput").ap()
    keys = nc.dram_tensor("keys", [N_BLOCKS, DK], F32, kind="ExternalInput").ap()
    att = nc.dram_tensor("att", [N_BLOCKS, 1], F32, kind="ExternalOutput").ap()

    with tile.TileContext(nc) as tc, ExitStack() as ctx:
        kn_pool = ctx.enter_context(tc.tile_pool(name="kn", bufs=6))
        kt_pool = ctx.enter_context(tc.tile_pool(name="kt", bufs=2))
        small = ctx.enter_context(tc.tile_pool(name="small", bufs=1))
        pt_pool = ctx.enter_context(tc.tile_pool(name="pt", bufs=5, space="PSUM"))
        ps_pool = ctx.enter_context(tc.tile_pool(name="ps", bufs=2, space="PSUM"))
        aux_pool = ctx.enter_context(tc.tile_pool(name="aux", bufs=1, space="PSUM"))

        # tiny setup DMA first (query on the scalar ring), then keys
        # prefetch so the SDMA engines stream keys from the start
        qsb = small.tile([128, 8 * L], F32, tag="qsb")
        nc.scalar.dma_start(qsb[:], qp)
        ident = small.tile([128, 128], F32, tag="ident")
        masks.make_identity(nc, ident[:])

        kns = {}

        def load_group(g, split=1):
            kn = kn_pool.tile([128, 4 * H], F32, tag="kn")
            dma_eng = nc.sync if g % 2 == 0 else nc.scalar
            src = keys[8 * g : 8 * g + 8, :].rearrange(
                "n (jh jl h) -> (n jh) jl h", jl=4, h=H
            )
            dst = kn[:].rearrange("p (jl h) -> p jl h", h=H)
            step = 4 // split
            for i in range(split):
                dma_eng.dma_start(
                    dst[:, i * step : (i + 1) * step],
                    src[:, i * step : (i + 1) * step],
                )
            kns[g] = kn

        PREFETCH = 4
        # split the first two groups so PE transposes can start on the
        # first quarter instead of waiting for a full 2MB tile
        load_group(0, split=4)
        load_group(1, split=2)
        for g in range(2, PREFETCH):
            load_group(g)

        qTr = small.tile([128, 8 * L], F32R, tag="qTr")
        nc.vector.tensor_copy(qTr[:], qsb[:])

        # per-i accumulated block maxima [64 i, 128 n]
        att_acc = small.tile([L, N_BLOCKS], F32, tag="att_acc")

        for g in range(GROUPS):
            if g + PREFETCH < GROUPS:
                load_group(g + PREFETCH)
            kn = kns.pop(g)

            s_ps = ps_pool.tile([L, 512], F32, tag="s")
            kt = kt_pool.tile([128, 8 * 512], F32R, tag="kt")
            for c in range(8):  # h-chunks
                pt = pt_pool.tile([128, 512], F32, tag="pt")
                for l in range(4):
                    # transpose kn[(n j_hi), (j_lo=l, h-chunk c)]
                    nc.tensor.matmul(
                        pt[:, 128 * l : 128 * (l + 1)],
                        kn[:, 1024 * l + 128 * c : 1024 * l + 128 * (c + 1)],
                        ident[:],
                        is_transpose=True,
                    )
                # evict one bank (contiguous copy, rounds to f32r)
                if c % 2 == 0:
                    nc.vector.tensor_copy(kt[:, 512 * c : 512 * (c + 1)], pt[:])
                else:
                    nc.scalar.copy(kt[:, 512 * c : 512 * (c + 1)], pt[:])
            # scoring burst: 8 accumulating matmuls back-to-back
            for c in range(8):
                nc.tensor.matmul(
                    s_ps[:],
                    qTr[:, L * c : L * (c + 1)],
                    kt[:, 512 * c : 512 * (c + 1)],
                    start=(c == 0),
                    stop=(c == 7),
                )

            # s_ps columns are (l 4, n 8, j_hi 16): reduce max over j_hi,
            # then over l, keeping the 8 n-blocks
            red1 = small.tile([L, 32], F32, tag="red1")
            nc.vector.reduce_max(
                red1[:],
                s_ps[:].rearrange("i (ln j) -> i ln j", j=16),
                axis=mybir.AxisListType.X,
            )
            nc.vector.reduce_max(
                att_acc[:, 8 * g : 8 * (g + 1)],
                red1[:].rearrange("i (l n) -> i n l", l=4),
                axis=mybir.AxisListType.X,
            )

        # final: transpose [64 i, 128 n] -> [128 n, 64 i], max over i
        pfin = aux_pool.tile([128, L], F32, tag="aux")
        nc.tensor.matmul(
            pfin[:], att_acc[:], ident[0:L, 0:L], is_transpose=True
        )
        fin = small.tile([N_BLOCKS, 1], F32, tag="fin")
        nc.vector.reduce_max(fin[:], pfin[:], axis=mybir.AxisListType.X)
        nc.sync.dma_start(att, fin[:])

    nc.compile()
    return nc


def _get_nc():
    if "nc" not in _CACHE:
        _CACHE["nc"] = _build()
    return _CACHE["nc"]


def kernel(query: np.ndarray, keys: np.ndarray, values: np.ndarray):
    from concourse import bass_utils

    assert query.shape == (1, L, N_CORES, H)
    assert keys.shape == (N_BLOCKS, N_CORES, DK)

    nc = _get_nc()

    in_maps = []
    for b in range(N_CORES):
        qb = query[0, :, b, :]  # [L, H]
        # qp[p, c*64+i] = qb[i, 128c+p]
        qpk = np.ascontiguousarray(
            qb.T.reshape(8, 128, L).transpose(1, 0, 2).reshape(128, 8 * L),
            dtype=np.float32,
        )
        kb = np.ascontiguousarray(keys[:, b, :], dtype=np.float32)  # [N, DK]
        in_maps.append({"qp": qpk, "keys": kb})

    res = bass_utils.run_bass_kernel_spmd(
        nc, in_maps, core_ids=list(range(N_CORES)), **_CACHE.get("run_kwargs", {})
    )
    _CACHE["last_result"] = res

    att = np.empty((N_CORES, 1, N_BLOCKS), dtype=np.float32)
    for b in range(N_CORES):
        att[b, 0, :] = res.results[b]["att"][:, 0]

    # exact top-k: re-rank top candidate blocks in fp64 from raw inputs
    topk = np.empty((TOPK, N_CORES), dtype=np.int32)
    for b in range(N_CORES):
        cand = np.argsort(-att[b, 0], kind="stable")[:TOPC]
        qb = query[0, :, b, :].astype(np.float64)  # [64, 1024]
        kb = keys[cand, b, :].reshape(TOPC, L, H).astype(np.float64)
        # scores[n] = max_{i,j} q[i] . k[n, j]
        s = np.einsum("ih,njh->nij", qb, kb, optimize=True)
        sc = s.reshape(TOPC, -1).max(axis=1)
        order = np.argsort(-sc, kind="stable")[:TOPK]
        topk[:, b] = cand[order].astype(np.int32)

    return att, topk


# revision 21
# speedup vs baseline: 1.1947x; 1.0987x over previous
"""Trainium2 Bass kernel for nn_Cache_68135361184561 (retrieval_knn).

Computation (per batch element b, bsz=8):
    q_b   = query[0, :, b, :]                      # [L=64, h=1024]
    k_b   = keys[:, b, :].reshape(128, 64, 1024)   # [N, L, h]
    att[b, n] = max_{i,j} q_b[i] . k_b[n, j]       # [128]
    topk_idx  = top-8 blocks by att

values (512 MB) is unused by the reference computation.

Sharding: batch b -> NeuronCore b (8 cores, fully batch-parallel).

Device kernel (per core), 16 groups of 8 n-blocks each:
  - one 2MB DMA per group; partition p = (n_oct, j_hi) so each partition
    reads 16KB contiguous HBM (4 consecutive j-rows) -> fat descriptors
  - PE-transposes 128x128 chunks (exact fp32) into 2-bank PSUM tiles
  - DVE/ACT evict PSUM -> SBUF, reordering columns n-major and rounding
    to float32r (PE requirement for full-rate 4-byte matmul)
  - score S[i, j'] over 8 h-chunk matmuls (lhsT = qT chunk [128h, 64i],
    rhs = K^T [128h, 512] f32r, fp32 PSUM accumulate)
  - DVE max-reduce S per n-block, accumulate [64 i, 128 n]
  - final PE transpose + DVE max over i -> att [128 n]

Host: gathers per-core att, re-ranks top-16 candidate blocks in fp64 from
the raw inputs to produce exact top-8 indices.
"""
from contextlib import ExitStack

import numpy as np

TOPK = 8
TOPC = 16  # candidate blocks re-ranked on host in fp64
N_CORES = 8
L = 64
H = 1024
N_BLOCKS = 128
DK = L * H  # 65536
GROUPS = 16  # each group covers 8 n-blocks

_CACHE = {}


def _build():
    import concourse.bacc as bacc
    import concourse.tile as tile
    import concourse.mybir as mybir
    from concourse import masks

    F32 = mybir.dt.float32
    F32R = mybir.dt.float32r

    nc = bacc.Bacc("TRN2", target_bir_lowering=False, debug=False)
    # qp[p, c*64+i] = query[i, 128*c + p] (host-packed transposed query)
    qp = nc.dram_tensor("qp", [128, 8 * L], F32, kind="ExternalInput").ap()
    keys = nc.dram_tensor("keys", [N_BLOCKS, DK], F32, kind="ExternalInput").ap()
    att = nc.dram_tensor("att", [N_BLOCKS, 1], F32, kind="ExternalOutput").ap()

    with tile.TileContext(nc) as tc, ExitStack() as ctx:
        kn_pool = ctx.enter_context(tc.tile_pool(name="kn", bufs=6))
        kt_pool = ctx.enter_context(tc.tile_pool(name="kt", bufs=2))
        small = ctx.enter_context(tc.tile_pool(name="small", bufs=1))
        pt_pool = ctx.enter_context(tc.tile_pool(name="pt", bufs=5, space="PSUM"))
        ps_pool = ctx.enter_context(tc.tile_pool(name="ps", bufs=2, space="PSUM"))
        aux_pool = ctx.enter_context(tc.tile_pool(name="aux", bufs=1, space="PSUM"))

        # tiny setup DMA first (query on the scalar ring), then keys
        # prefetch so the SDMA engines stream keys from the start
        qsb = small.tile([128, 8 * L], F32, tag="qsb")
        nc.scalar.dma_start(qsb[:], qp)
        ident = small.tile([128, 128], F32, tag="ident")
        masks.make_identity(nc, ident[:])

        kns = {}

        def load_group(g, split=False):
            src = keys[8 * g : 8 * g + 8, :].rearrange(
                "n (jh jl h) -> (n jh) jl h", jl=4, h=H
            )
            if split:
                # four separate quarter tiles -> PE can start on the first
                # quarter as soon as it lands (Tile deps are tile-granular)
                parts = []
                for i in range(4):
                    knq = kn_pool.tile(
                        [128, H], F32, tag="knq", name=f"knq{g}_{i}", bufs=8
                    )
                    nc.sync.dma_start(knq[:], src[:, i])
                    parts.append(knq)
                kns[g] = parts
            else:
                kn = kn_pool.tile([128, 4 * H], F32, tag="kn")
                nc.sync.dma_start(kn[:].rearrange("p (jl h) -> p jl h", h=H), src)
                kns[g] = kn

        PREFETCH = 4
        # quarter the first two groups so PE transposes start early
        load_group(0, split=True)
        load_group(1, split=True)
        for g in range(2, PREFETCH):
            load_group(g)

        qTr = small.tile([128, 8 * L], F32R, tag="qTr")
        nc.vector.tensor_copy(qTr[:], qsb[:])

        # per-i accumulated block maxima [64 i, 128 n]
        att_acc = small.tile([L, N_BLOCKS], F32, tag="att_acc")

        for g in range(GROUPS):
            if g + PREFETCH < GROUPS:
                load_group(g + PREFETCH)
            kn = kns.pop(g)

            s_ps = ps_pool.tile([L, 512], F32, tag="s")
            kt = kt_pool.tile([128, 8 * 512], F32R, tag="kt")
            for c in range(8):  # h-chunks
                pt = pt_pool.tile([128, 512], F32, tag="pt")
                for l in range(4):
                    # transpose kn[(n j_hi), (j_lo=l, h-chunk c)]
                    if isinstance(kn, list):
                        src_t = kn[l][:, 128 * c : 128 * (c + 1)]
                    else:
                        src_t = kn[:, 1024 * l + 128 * c : 1024 * l + 128 * (c + 1)]
                    nc.tensor.matmul(
                        pt[:, 128 * l : 128 * (l + 1)],
                        src_t,
                        ident[:],
                        is_transpose=True,
                    )
                # evict one bank (contiguous copy, rounds to f32r)
                if c % 2 == 0:
                    nc.vector.tensor_copy(kt[:, 512 * c : 512 * (c + 1)], pt[:])
                else:
                    nc.scalar.copy(kt[:, 512 * c : 512 * (c + 1)], pt[:])
            # scoring burst: 8 accumulating matmuls back-to-back
            for c in range(8):
                nc.tensor.matmul(
                    s_ps[:],
                    qTr[:, L * c : L * (c + 1)],
                    kt[:, 512 * c : 512 * (c + 1)],
                    start=(c == 0),
                    stop=(c == 7),
                )

            # s_ps columns are (l 4, n 8, j_hi 16): reduce max over j_hi,
            # then over l, keeping the 8 n-blocks
            red1 = small.tile([L, 32], F32, tag="red1")
            nc.vector.reduce_max(
                red1[:],
                s_ps[:].rearrange("i (ln j) -> i ln j", j=16),
                axis=mybir.AxisListType.X,
            )
            nc.vector.reduce_max(
                att_acc[:, 8 * g : 8 * (g + 1)],
                red1[:].rearrange("i (l n) -> i n l", l=4),
                axis=mybir.AxisListType.X,
            )

        # final: transpose [64 i, 128 n] -> [128 n, 64 i], max over i
        pfin = aux_pool.tile([128, L], F32, tag="aux")
        nc.tensor.matmul(
            pfin[:], att_acc[:], ident[0:L, 0:L], is_transpose=True
        )
        fin = small.tile([N_BLOCKS, 1], F32, tag="fin")
        nc.vector.reduce_max(fin[:], pfin[:], axis=mybir.AxisListType.X)
        nc.sync.dma_start(att, fin[:])

    nc.compile()
    return nc


def _get_nc():
    if "nc" not in _CACHE:
        _CACHE["nc"] = _build()
    return _CACHE["nc"]


def kernel(query: np.ndarray, keys: np.ndarray, values: np.ndarray):
    from concourse import bass_utils

    assert query.shape == (1, L, N_CORES, H)
    assert keys.shape == (N_BLOCKS, N_CORES, DK)

    nc = _get_nc()

    in_maps = []
    for b in range(N_CORES):
        qb = query[0, :, b, :]  # [L, H]
        # qp[p, c*64+i] = qb[i, 128c+p]
        qpk = np.ascontiguousarray(
            qb.T.reshape(8, 128, L).transpose(1, 0, 2).reshape(128, 8 * L),
            dtype=np.float32,
        )
        kb = np.ascontiguousarray(keys[:, b, :], dtype=np.float32)  # [N, DK]
        in_maps.append({"qp": qpk, "keys": kb})

    res = bass_utils.run_bass_kernel_spmd(
        nc, in_maps, core_ids=list(range(N_CORES)), **_CACHE.get("run_kwargs", {})
    )
    _CACHE["last_result"] = res

    att = np.empty((N_CORES, 1, N_BLOCKS), dtype=np.float32)
    for b in range(N_CORES):
        att[b, 0, :] = res.results[b]["att"][:, 0]

    # exact top-k: re-rank top candidate blocks in fp64 from raw inputs
    topk = np.empty((TOPK, N_CORES), dtype=np.int32)
    for b in range(N_CORES):
        cand = np.argsort(-att[b, 0], kind="stable")[:TOPC]
        qb = query[0, :, b, :].astype(np.float64)  # [64, 1024]
        kb = keys[cand, b, :].reshape(TOPC, L, H).astype(np.float64)
        # scores[n] = max_{i,j} q[i] . k[n, j]
        s = np.einsum("ih,njh->nij", qb, kb, optimize=True)
        sc = s.reshape(TOPC, -1).max(axis=1)
        order = np.argsort(-sc, kind="stable")[:TOPK]
        topk[:, b] = cand[order].astype(np.int32)

    return att, topk


# revision 22
# speedup vs baseline: 1.2656x; 1.0593x over previous
"""Trainium2 Bass kernel for nn_Cache_68135361184561 (retrieval_knn).

Computation (per batch element b, bsz=8):
    q_b   = query[0, :, b, :]                      # [L=64, h=1024]
    k_b   = keys[:, b, :].reshape(128, 64, 1024)   # [N, L, h]
    att[b, n] = max_{i,j} q_b[i] . k_b[n, j]       # [128]
    topk_idx  = top-8 blocks by att

values (512 MB) is unused by the reference computation.

Sharding: batch b -> NeuronCore b (8 cores, fully batch-parallel).

Device kernel (per core), 16 groups of 8 n-blocks each:
  - one 2MB DMA per group; partition p = (n_oct, j_hi) so each partition
    reads 16KB contiguous HBM (4 consecutive j-rows) -> fat descriptors
  - PE-transposes 128x128 chunks (exact fp32) into 2-bank PSUM tiles
  - DVE/ACT evict PSUM -> SBUF, reordering columns n-major and rounding
    to float32r (PE requirement for full-rate 4-byte matmul)
  - score S[i, j'] over 8 h-chunk matmuls (lhsT = qT chunk [128h, 64i],
    rhs = K^T [128h, 512] f32r, fp32 PSUM accumulate)
  - DVE max-reduce S per n-block, accumulate [64 i, 128 n]
  - final PE transpose + DVE max over i -> att [128 n]

Host: gathers per-core att, re-ranks top-16 candidate blocks in fp64 from
the raw inputs to produce exact top-8 indices.
"""
from contextlib import ExitStack

import numpy as np

TOPK = 8
TOPC = 16  # candidate blocks re-ranked on host in fp64
N_CORES = 8
L = 64
H = 1024
N_BLOCKS = 128
DK = L * H  # 65536
GROUPS = 16  # each group covers 8 n-blocks

_CACHE = {}


def _build():
    import concourse.bacc as bacc
    import concourse.tile as tile
    import concourse.mybir as mybir
    from concourse import masks

    F32 = mybir.dt.float32
    F32R = mybir.dt.float32r

    nc = bacc.Bacc("TRN2", target_bir_lowering=False, debug=False)
    # qp[p, c*64+i] = query[i, 128*c + p] (host-packed transposed query)
    qp = nc.dram_tensor("qp", [128, 8 * L], F32, kind="ExternalInput").ap()
    keys = nc.dram_tensor("keys", [N_BLOCKS, DK], F32, kind="ExternalInput").ap()
    att = nc.dram_tensor("att", [L, N_BLOCKS], F32, kind="ExternalOutput").ap()

    with tile.TileContext(nc) as tc, ExitStack() as ctx:
        kn_pool = ctx.enter_context(tc.tile_pool(name="kn", bufs=5))
        kt_pool = ctx.enter_context(tc.tile_pool(name="kt", bufs=3))
        small = ctx.enter_context(tc.tile_pool(name="small", bufs=1))
        pt_pool = ctx.enter_context(tc.tile_pool(name="pt", bufs=5, space="PSUM"))
        ps_pool = ctx.enter_context(tc.tile_pool(name="ps", bufs=2, space="PSUM"))
        aux_pool = ctx.enter_context(tc.tile_pool(name="aux", bufs=1, space="PSUM"))

        # tiny setup DMA first (query on the scalar ring), then keys
        # prefetch so the SDMA engines stream keys from the start
        qsb = small.tile([128, 8 * L], F32, tag="qsb")
        nc.scalar.dma_start(qsb[:], qp)
        ident = small.tile([128, 128], F32, tag="ident")
        masks.make_identity(nc, ident[:])

        kns = {}

        def load_group(g, split=False):
            src = keys[8 * g : 8 * g + 8, :].rearrange(
                "n (jh jl h) -> (n jh) jl h", jl=4, h=H
            )
            if split:
                # four separate quarter tiles -> PE can start on the first
                # quarter as soon as it lands (Tile deps are tile-granular)
                parts = []
                for i in range(4):
                    knq = kn_pool.tile(
                        [128, H], F32, tag="knq", name=f"knq{g}_{i}", bufs=8
                    )
                    nc.sync.dma_start(knq[:], src[:, i])
                    parts.append(knq)
                kns[g] = parts
            else:
                kn = kn_pool.tile([128, 4 * H], F32, tag="kn")
                nc.sync.dma_start(kn[:].rearrange("p (jl h) -> p jl h", h=H), src)
                kns[g] = kn

        PREFETCH = 4
        # quarter the first two groups so PE transposes start early
        load_group(0, split=True)
        load_group(1, split=True)
        for g in range(2, PREFETCH):
            load_group(g)

        qTr = small.tile([128, 8 * L], F32R, tag="qTr")
        nc.vector.tensor_copy(qTr[:], qsb[:])

        # per-i accumulated block maxima [64 i, 128 n]
        att_acc = small.tile([L, N_BLOCKS], F32, tag="att_acc")

        for g in range(GROUPS):
            if g + PREFETCH < GROUPS:
                load_group(g + PREFETCH)
            kn = kns.pop(g)

            s_ps = ps_pool.tile([L, 512], F32, tag="s")
            kt = kt_pool.tile([128, 8 * 512], F32R, tag="kt")
            for c in range(8):  # h-chunks
                pt = pt_pool.tile([128, 512], F32, tag="pt")
                for l in range(4):
                    # transpose kn[(n j_hi), (j_lo=l, h-chunk c)]
                    if isinstance(kn, list):
                        src_t = kn[l][:, 128 * c : 128 * (c + 1)]
                    else:
                        src_t = kn[:, 1024 * l + 128 * c : 1024 * l + 128 * (c + 1)]
                    nc.tensor.matmul(
                        pt[:, 128 * l : 128 * (l + 1)],
                        src_t,
                        ident[:],
                        is_transpose=True,
                    )
                # evict one bank (contiguous copy, rounds to f32r)
                if c % 2 == 0:
                    nc.vector.tensor_copy(kt[:, 512 * c : 512 * (c + 1)], pt[:])
                else:
                    nc.scalar.copy(kt[:, 512 * c : 512 * (c + 1)], pt[:])
            # scoring burst: 8 accumulating matmuls back-to-back
            for c in range(8):
                nc.tensor.matmul(
                    s_ps[:],
                    qTr[:, L * c : L * (c + 1)],
                    kt[:, 512 * c : 512 * (c + 1)],
                    start=(c == 0),
                    stop=(c == 7),
                )

            # s_ps columns are (l 4, n 8, j_hi 16): reduce max over j_hi,
            # then over l, keeping the 8 n-blocks
            red1 = small.tile([L, 32], F32, tag="red1")
            nc.vector.reduce_max(
                red1[:],
                s_ps[:].rearrange("i (ln j) -> i ln j", j=16),
                axis=mybir.AxisListType.X,
            )
            nc.vector.reduce_max(
                att_acc[:, 8 * g : 8 * (g + 1)],
                red1[:].rearrange("i (l n) -> i n l", l=4),
                axis=mybir.AxisListType.X,
            )

        # ship [64 i, 128 n]; host does the final (exact) max over i
        nc.sync.dma_start(att, att_acc[:])

    nc.compile()
    return nc


def _get_nc():
    if "nc" not in _CACHE:
        _CACHE["nc"] = _build()
    return _CACHE["nc"]


def kernel(query: np.ndarray, keys: np.ndarray, values: np.ndarray):
    from concourse import bass_utils

    assert query.shape == (1, L, N_CORES, H)
    assert keys.shape == (N_BLOCKS, N_CORES, DK)

    nc = _get_nc()

    in_maps = []
    for b in range(N_CORES):
        qb = query[0, :, b, :]  # [L, H]
        # qp[p, c*64+i] = qb[i, 128c+p]
        qpk = np.ascontiguousarray(
            qb.T.reshape(8, 128, L).transpose(1, 0, 2).reshape(128, 8 * L),
            dtype=np.float32,
        )
        kb = np.ascontiguousarray(keys[:, b, :], dtype=np.float32)  # [N, DK]
        in_maps.append({"qp": qpk, "keys": kb})

    res = bass_utils.run_bass_kernel_spmd(
        nc, in_maps, core_ids=list(range(N_CORES)), **_CACHE.get("run_kwargs", {})
    )
    _CACHE["last_result"] = res

    att = np.empty((N_CORES, 1, N_BLOCKS), dtype=np.float32)
    for b in range(N_CORES):
        att[b, 0, :] = res.results[b]["att"].max(axis=0)

    # exact top-k: re-rank top candidate blocks in fp64 from raw inputs
    topk = np.empty((TOPK, N_CORES), dtype=np.int32)
    for b in range(N_CORES):
        cand = np.argsort(-att[b, 0], kind="stable")[:TOPC]
        qb = query[0, :, b, :].astype(np.float64)  # [64, 1024]
        kb = keys[cand, b, :].reshape(TOPC, L, H).astype(np.float64)
        # scores[n] = max_{i,j} q[i] . k[n, j]
        s = np.einsum("ih,njh->nij", qb, kb, optimize=True)
        sc = s.reshape(TOPC, -1).max(axis=1)
        order = np.argsort(-sc, kind="stable")[:TOPK]
        topk[:, b] = cand[order].astype(np.int32)

    return att, topk


# revision 23
# speedup vs baseline: 1.3794x; 1.0900x over previous
"""Trainium2 Bass kernel for nn_Cache_68135361184561 (retrieval_knn).

Computation (per batch element b, bsz=8):
    q_b   = query[0, :, b, :]                      # [L=64, h=1024]
    k_b   = keys[:, b, :].reshape(128, 64, 1024)   # [N, L, h]
    att[b, n] = max_{i,j} q_b[i] . k_b[n, j]       # [128]
    topk_idx  = top-8 blocks by att

values (512 MB) is unused by the reference computation.

Sharding: batch b -> NeuronCore b (8 cores, fully batch-parallel).

Device kernel (per core), 16 groups of 8 n-blocks each:
  - one 2MB DMA per group; partition p = (n_oct, j_hi) so each partition
    reads 16KB contiguous HBM (4 consecutive j-rows) -> fat descriptors
  - PE-transposes 128x128 chunks (exact fp32) into 2-bank PSUM tiles
  - DVE/ACT evict PSUM -> SBUF, reordering columns n-major and rounding
    to float32r (PE requirement for full-rate 4-byte matmul)
  - score S[i, j'] over 8 h-chunk matmuls (lhsT = qT chunk [128h, 64i],
    rhs = K^T [128h, 512] f32r, fp32 PSUM accumulate)
  - DVE max-reduce S per n-block, accumulate [64 i, 128 n]
  - final PE transpose + DVE max over i -> att [128 n]

Host: gathers per-core att, re-ranks top-16 candidate blocks in fp64 from
the raw inputs to produce exact top-8 indices.
"""
from contextlib import ExitStack

import numpy as np

TOPK = 8
TOPC = 16  # candidate blocks re-ranked on host in fp64
N_CORES = 8
L = 64
H = 1024
N_BLOCKS = 128
DK = L * H  # 65536
GROUPS = 16  # each group covers 8 n-blocks

_CACHE = {}


def _build():
    import concourse.bacc as bacc
    import concourse.tile as tile
    import concourse.mybir as mybir

    F32 = mybir.dt.float32
    F32R = mybir.dt.float32r

    nc = bacc.Bacc("TRN2", target_bir_lowering=False, debug=False)
    # qp[p, c*64+i] = query[i, 128*c + p] (host-packed transposed query)
    qp = nc.dram_tensor("qp", [128, 8 * L], F32, kind="ExternalInput").ap()
    idin = nc.dram_tensor("idin", [128, 128], F32, kind="ExternalInput").ap()
    keys = nc.dram_tensor("keys", [N_BLOCKS, DK], F32, kind="ExternalInput").ap()
    att = nc.dram_tensor("att", [L, N_BLOCKS], F32, kind="ExternalOutput").ap()

    with tile.TileContext(nc) as tc, ExitStack() as ctx:
        kn_pool = ctx.enter_context(tc.tile_pool(name="kn", bufs=5))
        kt_pool = ctx.enter_context(tc.tile_pool(name="kt", bufs=3))
        small = ctx.enter_context(tc.tile_pool(name="small", bufs=1))
        pt_pool = ctx.enter_context(tc.tile_pool(name="pt", bufs=5, space="PSUM"))
        ps_pool = ctx.enter_context(tc.tile_pool(name="ps", bufs=2, space="PSUM"))
        aux_pool = ctx.enter_context(tc.tile_pool(name="aux", bufs=1, space="PSUM"))

        # tiny setup DMA first (query on the scalar ring), then keys
        # prefetch so the SDMA engines stream keys from the start
        qsb = small.tile([128, 8 * L], F32, tag="qsb")
        nc.scalar.dma_start(qsb[:], qp)
        ident = small.tile([128, 128], F32, tag="ident")
        nc.scalar.dma_start(ident[:], idin)

        kns = {}

        def load_group(g, split=False):
            src = keys[8 * g : 8 * g + 8, :].rearrange(
                "n (jh jl h) -> (n jh) jl h", jl=4, h=H
            )
            if split:
                # four separate quarter tiles -> PE can start on the first
                # quarter as soon as it lands (Tile deps are tile-granular)
                parts = []
                for i in range(4):
                    knq = kn_pool.tile(
                        [128, H], F32, tag="knq", name=f"knq{g}_{i}", bufs=8
                    )
                    nc.sync.dma_start(knq[:], src[:, i])
                    parts.append(knq)
                kns[g] = parts
            else:
                kn = kn_pool.tile([128, 4 * H], F32, tag="kn")
                nc.sync.dma_start(kn[:].rearrange("p (jl h) -> p jl h", h=H), src)
                kns[g] = kn

        PREFETCH = 4
        # quarter the first two groups so PE transposes start early
        load_group(0, split=True)
        load_group(1, split=True)
        for g in range(2, PREFETCH):
            load_group(g)

        qTr = small.tile([128, 8 * L], F32R, tag="qTr")
        nc.vector.tensor_copy(qTr[:], qsb[:])

        # per-i accumulated block maxima [64 i, 128 n]
        att_acc = small.tile([L, N_BLOCKS], F32, tag="att_acc")

        for g in range(GROUPS):
            if g + PREFETCH < GROUPS:
                load_group(g + PREFETCH)
            kn = kns.pop(g)

            s_ps = ps_pool.tile([L, 512], F32, tag="s")
            kt = kt_pool.tile([128, 8 * 512], F32R, tag="kt")
            for c in range(8):  # h-chunks
                pt = pt_pool.tile([128, 512], F32, tag="pt")
                for l in range(4):
                    # transpose kn[(n j_hi), (j_lo=l, h-chunk c)]
                    if isinstance(kn, list):
                        src_t = kn[l][:, 128 * c : 128 * (c + 1)]
                    else:
                        src_t = kn[:, 1024 * l + 128 * c : 1024 * l + 128 * (c + 1)]
                    nc.tensor.matmul(
                        pt[:, 128 * l : 128 * (l + 1)],
                        src_t,
                        ident[:],
                        is_transpose=True,
                    )
                # evict one bank (contiguous copy, rounds to f32r)
                if c in (0, 3, 6):
                    nc.vector.tensor_copy(kt[:, 512 * c : 512 * (c + 1)], pt[:])
                else:
                    nc.scalar.copy(kt[:, 512 * c : 512 * (c + 1)], pt[:])
            # scoring burst: 8 accumulating matmuls back-to-back
            for c in range(8):
                nc.tensor.matmul(
                    s_ps[:],
                    qTr[:, L * c : L * (c + 1)],
                    kt[:, 512 * c : 512 * (c + 1)],
                    start=(c == 0),
                    stop=(c == 7),
                )

            # s_ps columns are (l 4, n 8, j_hi 16): reduce max over j_hi,
            # then over l, keeping the 8 n-blocks
            red1 = small.tile([L, 32], F32, tag="red1")
            nc.vector.reduce_max(
                red1[:],
                s_ps[:].rearrange("i (ln j) -> i ln j", j=16),
                axis=mybir.AxisListType.X,
            )
            nc.vector.reduce_max(
                att_acc[:, 8 * g : 8 * (g + 1)],
                red1[:].rearrange("i (l n) -> i n l", l=4),
                axis=mybir.AxisListType.X,
            )
            if g == 7:
                # ship the first half early; host does the final max over i
                nc.sync.dma_start(att[:, 0:64], att_acc[:, 0:64])

        nc.sync.dma_start(att[:, 64:128], att_acc[:, 64:128])

    nc.compile()
    return nc


def _get_nc():
    if "nc" not in _CACHE:
        _CACHE["nc"] = _build()
    return _CACHE["nc"]


def kernel(query: np.ndarray, keys: np.ndarray, values: np.ndarray):
    from concourse import bass_utils

    assert query.shape == (1, L, N_CORES, H)
    assert keys.shape == (N_BLOCKS, N_CORES, DK)

    nc = _get_nc()

    in_maps = []
    for b in range(N_CORES):
        qb = query[0, :, b, :]  # [L, H]
        # qp[p, c*64+i] = qb[i, 128c+p]
        qpk = np.ascontiguousarray(
            qb.T.reshape(8, 128, L).transpose(1, 0, 2).reshape(128, 8 * L),
            dtype=np.float32,
        )
        kb = np.ascontiguousarray(keys[:, b, :], dtype=np.float32)  # [N, DK]
        in_maps.append({"qp": qpk, "keys": kb, "idin": np.eye(128, dtype=np.float32)})

    res = bass_utils.run_bass_kernel_spmd(
        nc, in_maps, core_ids=list(range(N_CORES)), **_CACHE.get("run_kwargs", {})
    )
    _CACHE["last_result"] = res

    att = np.empty((N_CORES, 1, N_BLOCKS), dtype=np.float32)
    for b in range(N_CORES):
        att[b, 0, :] = res.results[b]["att"].max(axis=0)

    # exact top-k: re-rank top candidate blocks in fp64 from raw inputs
    topk = np.empty((TOPK, N_CORES), dtype=np.int32)
    for b in range(N_CORES):
        cand = np.argsort(-att[b, 0], kind="stable")[:TOPC]
        qb = query[0, :, b, :].astype(np.float64)  # [64, 1024]
        kb = keys[cand, b, :].reshape(TOPC, L, H).astype(np.float64)
        # scores[n] = max_{i,j} q[i] . k[n, j]
        s = np.einsum("ih,njh->nij", qb, kb, optimize=True)
        sc = s.reshape(TOPC, -1).max(axis=1)
        order = np.argsort(-sc, kind="stable")[:TOPK]
        topk[:, b] = cand[order].astype(np.int32)

    return att, topk
